# revision 53
# baseline (speedup 1.0000x reference)
"""H2GCN forward pass on 8 Trainium2 NeuronCores (Bass/Tile SPMD kernel).

Strategy (1D row-parallel SpMM, binary-factored fp8 adjacencies):
  - Nodes sharded across 8 cores (1024 rows each). Both gcn-normalized
    adjacencies factor exactly as A = D^-1/2 @ Abin @ D^-1/2 with Abin BINARY,
    so each core receives the column-slice Abin[:, rows] as fp8 (0/1 are exact
    in fp8) -- half the HBM traffic of fp16 values, zero quantization error on
    the matrices. The D^-1/2 factors move into the features (contraction side)
    and cheap broadcast multiplies (output side).
  - conv1 (z = [A@h; A2@h], raw): lhsT = fp16(dis*h) x fp8 binary rhs, mixed-
    dtype matmul at fp16 rate. Numerically a fp16-exact SpMM.
  - conv2 (u = [A@z; A2@z], raw): both operands fp8 with DoubleRow perf mode
    (2x PE throughput). Features are MEAN-CENTERED before quantization
    (A@z = A@(z-m) + rowsum (x) m, m = host-estimated column mean of z);
    without centering, fp8's relative error on the large post-ReLU means wipes
    out the post-BN signal. The rank-1 rowsum (x) m correction is folded into
    the PSUM drain. Net: fp8 conv2 error lands below the fp16 noise floor.
  - BatchNorm is algebraically absorbed into the final projection:
        z_n = z*c + d,  A@z_n = (A@z)*c + rowsum(A) (x) d
    so conv2 runs on raw z and the projection uses per-row scaled weights plus
    rank-1 corrections from host-provided exact rowsums.
  - Raw z is transposed, scaled into the two fp8 operand forms, and
    AllGathered (Shared-output HBM collective) in two feature halves: z1's
    gather hides under conv1's second half, z2's under conv2's first pass.
    conv2 streams the adjacencies twice (cheap in fp8). BN statistics use a
    tiny AllReduce off the critical path.
"""

import numpy as np
import ml_dtypes

import concourse.bass as bass
import concourse.mybir as mybir
import concourse.tile as tile
from concourse import bacc
from concourse.bass_utils import run_bass_kernel_spmd
from concourse.masks import make_identity

P = 128
NCORES = 8
BN_EPS = 1e-5

F8 = mybir.dt.float8e4
F16 = mybir.dt.float16
F32 = mybir.dt.float32
NP_F8 = ml_dtypes.float8_e4m3

FULL_CFG = dict(NT=8192, R=1024)
IN_CH = 512   # input features
H = 256       # hidden
H2 = 512      # 2*H (BN width)
O = 64        # output features
F = 7 * H     # 1792, JK concat width


def build_program(NT, R):
    """Build the SPMD Bass program. NT = total nodes, R = rows per core."""
    KT = NT // P           # node k-tiles (contraction tiles), 64
    DKT = KT // 2          # double k-tiles, 32
    RT = R // P            # per-core node tiles, 8
    NCH = [(0, 512), (512, 512)]  # (start, width) node chunks of R
    HM = H // P            # 2
    H2M = H2 // P          # 4
    FM = F // P            # 14
    INK = IN_CH // P       # 4

    nc = bacc.Bacc("TRN2", target_bir_lowering=False, debug=False,
                   num_devices=NCORES)

    # --- I/O -------------------------------------------------------------
    xTf = nc.dram_tensor("xTf", [IN_CH, NT], F16, kind="ExternalInput")
    xT = nc.dram_tensor("xT", [IN_CH, R], F16, kind="ExternalInput")
    adjT = nc.dram_tensor("adjT", [NT, R], F8, kind="ExternalInput")
    adjT2 = nc.dram_tensor("adjT2", [NT, R], F8, kind="ExternalInput")
    wTe = nc.dram_tensor("wTe", [IN_CH, H], F16, kind="ExternalInput")
    be = nc.dram_tensor("be", [P, HM], F32, kind="ExternalInput")
    bebc16 = nc.dram_tensor("bebc16", [P, H], F16, kind="ExternalInput")
    wTf = nc.dram_tensor("wTf", [F, O], F16, kind="ExternalInput")
    bff = nc.dram_tensor("bff", [O, 1], F32, kind="ExternalInput")
    gam = nc.dram_tensor("gam", [P, H2M], F32, kind="ExternalInput")
    bet = nc.dram_tensor("bet", [P, H2M], F32, kind="ExternalInput")
    edis1 = nc.dram_tensor("edis1", [P, KT], F32, kind="ExternalInput")
    edis2 = nc.dram_tensor("edis2", [P, KT], F32, kind="ExternalInput")
    zdis1 = nc.dram_tensor("zdis1", [P, RT], F32, kind="ExternalInput")
    zdis2 = nc.dram_tensor("zdis2", [P, RT], F32, kind="ExternalInput")
    mzneg = nc.dram_tensor("mzneg", [P, H2M], F32, kind="ExternalInput")
    mzpos = nc.dram_tensor("mzpos", [P, H2M], F32, kind="ExternalInput")
    dbc1 = nc.dram_tensor("dbc1", [P, R], F32, kind="ExternalInput")
    dbc2 = nc.dram_tensor("dbc2", [P, R], F32, kind="ExternalInput")
    ubc1 = nc.dram_tensor("ubc1", [P, R], F32, kind="ExternalInput")
    ubc2 = nc.dram_tensor("ubc2", [P, R], F32, kind="ExternalInput")
    rs1bc = nc.dram_tensor("rs1bc", [P, R], F32, kind="ExternalInput")
    rs2bc = nc.dram_tensor("rs2bc", [P, R], F32, kind="ExternalInput")
    rsA = nc.dram_tensor("rsA", [O, R], F32, kind="ExternalInput")
    rsA2 = nc.dram_tensor("rsA2", [O, R], F32, kind="ExternalInput")
    out = nc.dram_tensor("out", [R, O], F32, kind="ExternalOutput")

    rg = [list(range(NCORES))]

    with tile.TileContext(nc) as tc:
        with (
            tc.tile_pool(name="const", bufs=1) as const,
            tc.tile_pool(name="feat", bufs=1) as feat,
            tc.tile_pool(name="tmp", bufs=2) as tmp,
            tc.tile_pool(name="stream", bufs=4) as stream,
            tc.tile_pool(name="ps", bufs=1, space="PSUM") as ps,
            tc.tile_pool(name="dram", bufs=1, space="DRAM") as dram,
        ):
            # --- constants / weights (embed-critical ones first) --------
            wTe_sb = const.tile([P, INK, H], F16, name="wTe_sb")
            nc.sync.dma_start(wTe_sb[:], wTe.ap().rearrange("(k p) m -> p k m", p=P))
            edis1_sb = const.tile([P, KT], F32, name="edis1_sb")
            nc.sync.dma_start(edis1_sb[:], edis1.ap())
            edis2_sb = const.tile([P, KT], F32, name="edis2_sb")
            nc.sync.dma_start(edis2_sb[:], edis2.ap())
            be_sb = const.tile([P, HM], F32, name="be_sb")
            nc.sync.dma_start(be_sb[:], be.ap())
            bebc16_sb = const.tile([P, H], F16, name="bebc16_sb")
            nc.sync.dma_start(bebc16_sb[:], bebc16.ap())
            ones_row = const.tile([P, P], F16, name="ones_row")
            nc.vector.memset(ones_row[:], 0.0)
            nc.vector.memset(ones_row[0:1, :], 1.0)
            id16 = const.tile([P, P], F16, name="id16")
            make_identity(nc, id16)

            # full x.T, loaded in node-chunk groups so the embed can start
            # before the whole 8 MB lands (all other constants come after)
            xTf_t = xTf.ap().rearrange("(k p) n -> p k n", p=P)
            xTf_sb = feat.tile([P, INK, NT], F16, name="xTf_sb", tag="big0")
            XGRP = max(NT // 16, P)
            for g in range(0, NT, XGRP):
                nc.sync.dma_start(xTf_sb[:, :, g:g + XGRP], xTf_t[:, :, g:g + XGRP])

            xT_sb = const.tile([P, INK, R], F16, name="xT_sb")
            nc.sync.dma_start(xT_sb[:], xT.ap().rearrange("(k p) n -> p k n", p=P))
            id32 = const.tile([P, P], F32, name="id32")
            make_identity(nc, id32)
            dbc1_sb = const.tile([P, R], F32, name="dbc1_sb", tag="bc1")
            nc.sync.dma_start(dbc1_sb[:], dbc1.ap())
            dbc2_sb = const.tile([P, R], F32, name="dbc2_sb", tag="bc2")
            nc.sync.dma_start(dbc2_sb[:], dbc2.ap())
            zdis1_sb = const.tile([P, RT], F32, name="zdis1_sb")
            nc.sync.dma_start(zdis1_sb[:], zdis1.ap())
            zdis2_sb = const.tile([P, RT], F32, name="zdis2_sb")
            nc.sync.dma_start(zdis2_sb[:], zdis2.ap())
            mzneg_sb = const.tile([P, H2M], F32, name="mzneg_sb")
            nc.sync.dma_start(mzneg_sb[:], mzneg.ap())
            mzpos_sb = const.tile([P, H2M], F32, name="mzpos_sb")
            nc.sync.dma_start(mzpos_sb[:], mzpos.ap())
            rs1bc_sb = const.tile([P, R], F32, name="rs1bc_sb")
            nc.gpsimd.dma_start(rs1bc_sb[:], rs1bc.ap())
            rs2bc_sb = const.tile([P, R], F32, name="rs2bc_sb")
            nc.gpsimd.dma_start(rs2bc_sb[:], rs2bc.ap())
            wTf_sb = const.tile([P, FM, O], F16, name="wTf_sb")
            nc.sync.dma_start(wTf_sb[:], wTf.ap().rearrange("(k p) m -> p k m", p=P))
            bff_sb = const.tile([O, 1], F32, name="bff_sb")
            nc.sync.dma_start(bff_sb[:], bff.ap())

            # --- phase B1: replicated full embed, node-major ------------
            # h16a[node, f] = dis1[node]*relu(x @ w_embed.T + b); h16b w/ dis2
            h16a = feat.tile([P, KT, H], F16, name="h16a", tag="big1a")
            h16b = feat.tile([P, KT, H], F16, name="h16b", tag="big1b")
            for k in range(KT):
                hps = ps.tile([P, H], F32, name=f"hps_{k}", tag=f"acc{k % 8}")
                for t in range(INK):
                    nc.tensor.matmul(
                        hps[:],
                        lhsT=xTf_sb[:, t, k * P:(k + 1) * P],
                        rhs=wTe_sb[:, t, :],
                        start=(t == 0), stop=False,
                    )
                # bias via a rank-1 matmul: ones_row selects bebc16 row 0
                nc.tensor.matmul(
                    hps[:], lhsT=ones_row[:], rhs=bebc16_sb[:],
                    start=False, stop=True,
                )
                nc.scalar.activation(
                    h16a[:, k, :], hps[:],
                    mybir.ActivationFunctionType.Relu,
                    scale=edis1_sb[:, k:k + 1])
                nc.vector.tensor_scalar(
                    out=h16b[:, k, :], in0=hps[:],
                    scalar1=0.0, scalar2=edis2_sb[:, k:k + 1],
                    op0=mybir.AluOpType.max, op1=mybir.AluOpType.mult)

            # --- phase B2: local embed, feature-major (for the JK concat)
            hT_sb = feat.tile([P, HM, R], F16, name="hT_sb")
            for m in range(HM):
                for ci, (cs, cw) in enumerate(NCH):
                    eps_t = ps.tile([P, 512], F32, name=f"eps_{m}_{ci}",
                                    tag=f"acc{(m * 2 + ci) % 8}")
                    for t in range(INK):
                        nc.tensor.matmul(
                            eps_t[:, :cw],
                            lhsT=wTe_sb[:, t, m * P:(m + 1) * P],
                            rhs=xT_sb[:, t, cs:cs + cw],
                            start=(t == 0), stop=(t == INK - 1),
                        )
                    nc.scalar.activation(
                        hT_sb[:, m, cs:cs + cw], eps_t[:, :cw],
                        mybir.ActivationFunctionType.Relu,
                        bias=be_sb[:, m:m + 1],
                    )

            # --- phase D: conv1, zT = [A@h; A2@h].T (raw), one adjacency
            # half at a time; mixed fp16 x fp8-binary matmuls.
            # After each half: center, scale to fp8 forms, AllGather.
            zT_sb = feat.tile([P, H2M, R], F16, name="zT_sb")
            zag_out = [None, None]
            zf8_tiles = {}
            for half, (src, h16x, dbc_sb) in enumerate(
                    ((adjT, h16a, dbc1_sb), (adjT2, h16b, dbc2_sb))):
                zps = {}
                for m in range(HM):
                    for ci in range(len(NCH)):
                        zps[(m, ci)] = ps.tile(
                            [P, 512], F32, name=f"zps_{half}_{m}_{ci}",
                            tag=f"acc{(half * 4 + m * 2 + ci) % 8}")
                for dk in range(DKT):
                    at = stream.tile([P, 2, R], F8, name=f"c1_{half}_{dk}",
                                     tag="adj", bufs=3)
                    nc.sync.dma_start(
                        at[:], src.ap()[dk * 2 * P:(dk + 1) * 2 * P, :]
                        .rearrange("(s p) n -> p s n", p=P))
                    for s in range(2):
                        k = dk * 2 + s
                        for m in range(HM):
                            for ci, (cs, cw) in enumerate(NCH):
                                nc.tensor.matmul(
                                    zps[(m, ci)][:, :cw],
                                    lhsT=h16x[:, k, m * P:(m + 1) * P],
                                    rhs=at[:, s, cs:cs + cw],
                                    start=(k == 0), stop=(k == KT - 1),
                                )
                for m in range(HM):
                    for ci, (cs, cw) in enumerate(NCH):
                        nc.vector.tensor_tensor(
                            out=zT_sb[:, half * HM + m, cs:cs + cw],
                            in0=zps[(m, ci)][:, :cw],
                            in1=dbc_sb[:, cs:cs + cw],
                            op=mybir.AluOpType.mult)

                # center this z half (per-feature mean, host-estimated),
                # transpose to node-major, write BOTH fp8 operand forms
                # (dis1- and dis2-scaled), and AllGather them as one tensor.
                z8cat = tmp.tile([P, RT, H2], F8, name=f"z8cat_{half}",
                                 bufs=1, tag="z8cat")
                zc16s = []
                for fi in range(HM):
                    ch = half * HM + fi
                    zc16 = tmp.tile([P, R], F16, name=f"zc16_{fi}",
                                    tag="zc16", bufs=2)
                    nc.vector.tensor_scalar_add(
                        zc16[:], zT_sb[:, ch, :], mzneg_sb[:, ch:ch + 1])
                    zc16s.append(zc16)
                for fi in range(HM):
                    zc16 = zc16s[fi]
                    for nt in range(RT):
                        tps = ps.tile(
                            [P, P], F16, name=f"ztp_{half}_{fi}_{nt}",
                            tag=f"acc{nt % 4}")
                        nc.tensor.transpose(
                            tps[:], zc16[:, nt * P:(nt + 1) * P], id16[:])
                        nc.vector.tensor_scalar_mul(
                            z8cat[:, nt, fi * P:(fi + 1) * P], tps[:],
                            zdis1_sb[:, nt:nt + 1])
                        nc.scalar.activation(
                            z8cat[:, nt, H + fi * P:H + (fi + 1) * P], tps[:],
                            mybir.ActivationFunctionType.Copy,
                            scale=zdis2_sb[:, nt:nt + 1])
                zin = dram.tile([P, RT, H2], F8, name=f"zag_in_{half}")
                nc.sync.dma_start(zin[:], z8cat[:])
                zout = dram.tile([NCORES, P, RT, H2], F8,
                                 name=f"zag_out_{half}", addr_space="Shared")
                nc.gpsimd.collective_compute(
                    "AllGather", mybir.AluOpType.bypass, replica_groups=rg,
                    ins=[zin.opt()], outs=[zout.opt()],
                )
                zag_out[half] = zout
                if half == 0:
                    # load the gathered z1 in 8 core-chunks immediately:
                    # the scalar queue is idle here, so the triggers fire
                    # as soon as the AllGather completes and conv2 pass 0
                    # starts without waiting on a bulk load.
                    zf8_tiles[0] = feat.tile([P, KT, H2], F8, name="zf8_0",
                                             tag="big0")
                    for r in range(NCORES):
                        nc.scalar.dma_start(
                            zf8_tiles[0][:, r * RT:(r + 1) * RT, :],
                            zout[r:r + 1].rearrange("o p nt f -> p (o nt) f"))

            # conv2 drain scalings reuse the dbc slots (disjoint lifetimes)
            ubc1_sb = const.tile([P, R], F32, name="ubc1_sb", tag="bc1")
            nc.gpsimd.dma_start(ubc1_sb[:], ubc1.ap())
            ubc2_sb = const.tile([P, R], F32, name="ubc2_sb", tag="bc2")
            nc.gpsimd.dma_start(ubc2_sb[:], ubc2.ap())

            # --- phase F: conv2 on raw z, fp8 DoubleRow, two passes -----
            # pass 0 (z features 0:256, from z1) only needs zag_out[0], so
            # it overlaps z2's AllGather; each pass streams both adjacencies.
            # BN stats + AllReduce + the zf8_1 load are emitted mid-pass-0
            # so the scalar DMA queue serves work in data-readiness order.
            stat_sb = tmp.tile([P, 2 * H2M], F32, name="stat_sb", bufs=1)
            stat_g = tmp.tile([P, 2 * H2M], F32, name="stat_g", bufs=1)
            u_sb = feat.tile([P, 2 * H2M, R], F16, name="u_sb", tag="big1b")
            for half in (0, 1):
                zf8 = zf8_tiles[half]
                ups = {}
                for a in (0, 1):
                    for m in range(HM):
                        for ci in range(len(NCH)):
                            ups[(a, m, ci)] = ps.tile(
                                [P, 512], F32, name=f"ups_{half}_{a}_{m}_{ci}",
                                tag=f"acc{(a * 4 + m * 2 + ci) % 8}")
                for dk in range(DKT):
                    at = stream.tile([P, 2, R], F8, name=f"c2a_{half}_{dk}",
                                     tag="adj", bufs=3)
                    nc.sync.dma_start(
                        at[:], adjT.ap()[dk * 2 * P:(dk + 1) * 2 * P, :]
                        .rearrange("(s p) n -> p s n", p=P))
                    at2 = stream.tile([P, 2, R], F8, name=f"c2b_{half}_{dk}",
                                      tag="adj2", bufs=3)
                    nc.scalar.dma_start(
                        at2[:], adjT2.ap()[dk * 2 * P:(dk + 1) * 2 * P, :]
                        .rearrange("(s p) n -> p s n", p=P))
                    if half == 0 and dk == 12:
                        # BN stats + AllReduce (need only zT; off-path)
                        for f in range(H2M):
                            sq = tmp.tile([P, R], F16, name="sq", tag="sq",
                                          bufs=1)
                            nc.scalar.activation(
                                sq[:], zT_sb[:, f, :],
                                mybir.ActivationFunctionType.Copy,
                                accum_out=stat_sb[:, f:f + 1])
                            sq2 = tmp.tile([P, R], F16, name="sq2", tag="sq",
                                           bufs=1)
                            nc.scalar.activation(
                                sq2[:], zT_sb[:, f, :],
                                mybir.ActivationFunctionType.Square,
                                accum_out=stat_sb[:, H2M + f:H2M + f + 1])
                        ar_in = dram.tile([P, 2 * H2M], F32, name="ar_in")
                        nc.scalar.dma_start(ar_in[:], stat_sb[:])
                        ar_out = dram.tile([P, 2 * H2M], F32, name="ar_out")
                        nc.gpsimd.collective_compute(
                            "AllReduce", mybir.AluOpType.add,
                            replica_groups=rg,
                            ins=[ar_in.opt()], outs=[ar_out.opt()],
                        )
                        nc.gpsimd.dma_start(stat_g[:], ar_out[:])
                    if half == 0 and dk == 24:
                        # queue the gathered-z2 load only after AG2 has
                        # completed, so these triggers never block the
                        # scalar queue (and the at2 feed behind them)
                        zf8_tiles[1] = feat.tile(
                            [P, KT, H2], F8, name="zf8_1", tag="big1a")
                        for r in range(NCORES):
                            nc.scalar.dma_start(
                                zf8_tiles[1][:, r * RT:(r + 1) * RT, :],
                                zag_out[1][r:r + 1].rearrange(
                                    "o p nt f -> p (o nt) f"))
                    for m in range(HM):
                        for ci, (cs, cw) in enumerate(NCH):
                            nc.tensor.matmul(
                                ups[(0, m, ci)][:, :cw],
                                lhsT=zf8[:, dk * 2:dk * 2 + 2,
                                         m * P:(m + 1) * P],
                                rhs=at[:, :, cs:cs + cw],
                                start=(dk == 0), stop=(dk == DKT - 1),
                                perf_mode=mybir.MatmulPerfMode.DoubleRow,
                            )
                            nc.tensor.matmul(
                                ups[(1, m, ci)][:, :cw],
                                lhsT=zf8[:, dk * 2:dk * 2 + 2,
                                         H + m * P:H + (m + 1) * P],
                                rhs=at2[:, :, cs:cs + cw],
                                start=(dk == 0), stop=(dk == DKT - 1),
                                perf_mode=mybir.MatmulPerfMode.DoubleRow,
                            )
                # u feature layout: chunks 0..3 = U1 (A@z), 4..7 = U2 (A2@z);
                # this pass produces z-feature chunks {half*2+m} of each.
                # u = psum*ubc + mz (x) rs  (undo dis/t scaling + centering)
                for a, (ubc_sb, rsbc_sb) in enumerate(
                        ((ubc1_sb, rs1bc_sb), (ubc2_sb, rs2bc_sb))):
                    for m in range(HM):
                        ch = half * HM + m
                        for ci, (cs, cw) in enumerate(NCH):
                            t32 = tmp.tile([P, 512], F32, name="t32",
                                           tag="t32", bufs=2)
                            nc.vector.tensor_tensor(
                                out=t32[:, :cw], in0=ups[(a, m, ci)][:, :cw],
                                in1=ubc_sb[:, cs:cs + cw],
                                op=mybir.AluOpType.mult)
                            t32b = tmp.tile([P, 512], F32, name="t32b",
                                            tag="t32b", bufs=2)
                            nc.vector.tensor_scalar_mul(
                                t32b[:, :cw], rsbc_sb[:, cs:cs + cw],
                                mzpos_sb[:, ch:ch + 1])
                            nc.vector.tensor_tensor(
                                out=u_sb[:, a * H2M + ch, cs:cs + cw],
                                in0=t32[:, :cw], in1=t32b[:, :cw],
                                op=mybir.AluOpType.add)

            # BN coefficients c, d (feature-major [128, 4], fp32).
            # Emitted after conv2 so no AllReduce-dependent op sits ahead
            # of conv2's work in any engine queue.
            gam_sb = const.tile([P, H2M], F32, name="gam_sb")
            nc.sync.dma_start(gam_sb[:], gam.ap())
            bet_sb = const.tile([P, H2M], F32, name="bet_sb")
            nc.sync.dma_start(bet_sb[:], bet.ap())
            cmean = tmp.tile([P, H2M], F32, name="cmean", bufs=1)
            nc.scalar.mul(cmean[:], stat_g[:, 0:H2M], 1.0 / NT)
            cvar = tmp.tile([P, H2M], F32, name="cvar", bufs=1)
            nc.scalar.mul(cvar[:], stat_g[:, H2M:2 * H2M], 1.0 / NT)
            msq = tmp.tile([P, H2M], F32, name="msq", bufs=1)
            nc.vector.tensor_mul(out=msq[:], in0=cmean[:], in1=cmean[:])
            nc.vector.tensor_tensor(
                out=cvar[:], in0=cvar[:], in1=msq[:],
                op=mybir.AluOpType.subtract)
            eps_sb = tmp.tile([P, 1], F32, name="eps_sb", bufs=1)
            nc.vector.memset(eps_sb[:], BN_EPS)
            cstd = tmp.tile([P, H2M], F32, name="cstd", bufs=1)
            nc.scalar.activation(
                cstd[:], cvar[:], mybir.ActivationFunctionType.Sqrt,
                bias=eps_sb[:])
            crstd = tmp.tile([P, H2M], F32, name="crstd", bufs=1)
            nc.vector.reciprocal(crstd[:], cstd[:])
            c_t = tmp.tile([P, H2M], F32, name="c_t", bufs=1)
            nc.vector.tensor_mul(out=c_t[:], in0=crstd[:], in1=gam_sb[:])
            d_t = tmp.tile([P, H2M], F32, name="d_t", bufs=1)
            nc.vector.tensor_mul(out=d_t[:], in0=cmean[:], in1=c_t[:])
            nc.vector.tensor_tensor(
                out=d_t[:], in0=bet_sb[:], in1=d_t[:],
                op=mybir.AluOpType.subtract)
            d16 = tmp.tile([P, H2M], F16, name="d16", bufs=1)
            nc.vector.tensor_copy(out=d16[:], in_=d_t[:])

            # --- phase G: final projection with absorbed BN -------------
            rsA_sb = const.tile([O, R], F32, name="rsA_sb", tag="bc1")
            nc.gpsimd.dma_start(rsA_sb[:], rsA.ap())
            rsA2_sb = const.tile([O, R], F32, name="rsA2_sb", tag="bc2")
            nc.gpsimd.dma_start(rsA2_sb[:], rsA2.ap())

            # s_j = W_block_j @ d  (blocks: z_n, U1, U2), from UNSCALED wTf
            s_cols = tmp.tile([O, 3], F32, name="s_cols", bufs=1)
            for j, base in enumerate((HM, HM + H2M, HM + 2 * H2M)):
                sps = ps.tile([O, 1], F32, name=f"sps_{j}", tag=f"acc{j}")
                for t in range(H2M):
                    nc.tensor.matmul(
                        sps[:], lhsT=wTf_sb[:, base + t, :],
                        rhs=d16[:, t:t + 1],
                        start=(t == 0), stop=(t == H2M - 1))
                nc.vector.tensor_copy(out=s_cols[:, j:j + 1], in_=sps[:])
            s0b = tmp.tile([O, 1], F32, name="s0b", bufs=1)
            nc.vector.tensor_add(out=s0b[:], in0=s_cols[:, 0:1], in1=bff_sb[:])

            # scale wTf rows (z_n, U1, U2 blocks) by c, in place
            for t in range(HM, FM):
                ch = (t - HM) % H2M
                nc.vector.tensor_scalar_mul(
                    wTf_sb[:, t, :], wTf_sb[:, t, :], c_t[:, ch:ch + 1])

            # outT[64, R] = wTf'.T @ jkT + (s0+bf) + s1 (x) rsA + s2 (x) rsA2
            def jk_rhs(t):
                if t < HM:
                    return hT_sb[:, t, :]
                if t < HM + H2M:
                    return zT_sb[:, t - HM, :]
                return u_sb[:, t - HM - H2M, :]

            outsb = tmp.tile([O, R], F32, name="outsb", bufs=1)
            for ci, (cs, cw) in enumerate(NCH):
                ops = ps.tile([O, 512], F32, name=f"ops_{ci}", tag=f"acc{4 + ci}")
                for t in range(FM):
                    nc.tensor.matmul(
                        ops[:, :cw], lhsT=wTf_sb[:, t, :],
                        rhs=jk_rhs(t)[:, cs:cs + cw],
                        start=(t == 0), stop=(t == FM - 1))
                nc.vector.tensor_scalar_add(
                    outsb[:, cs:cs + cw], ops[:, :cw], s0b[:])
            rk1 = feat.tile([O, R], F32, name="rk1", tag="big1a")
            nc.vector.tensor_scalar_mul(rk1[:], rsA_sb[:], s_cols[:, 1:2])
            nc.vector.tensor_add(out=outsb[:], in0=outsb[:], in1=rk1[:])
            rk2 = feat.tile([O, R], F32, name="rk2", tag="big1a")
            nc.vector.tensor_scalar_mul(rk2[:], rsA2_sb[:], s_cols[:, 2:3])
            nc.vector.tensor_add(out=outsb[:], in0=outsb[:], in1=rk2[:])

            # transpose [O, R] -> node-major [R, O] and write out
            o_nm = tmp.tile([P, RT, O], F32, name="o_nm", bufs=1)
            for nt in range(RT):
                tps32 = ps.tile([P, O], F32, name=f"otp_{nt}",
                                tag=f"acc{nt % 8}")
                nc.tensor.transpose(
                    tps32[:], outsb[:, nt * P:(nt + 1) * P], id32[:O, :O])
                nc.any.tensor_copy(out=o_nm[:, nt, :], in_=tps32[:])
            nc.sync.dma_start(
                out.ap().rearrange("(nt p) o -> p nt o", p=P), o_nm[:])

    nc.compile()
    return nc


_PROGRAM_CACHE = {}


def _get_program(NT, R):
    key = (NT, R)
    if key not in _PROGRAM_CACHE:
        _PROGRAM_CACHE[key] = build_program(NT, R)
    return _PROGRAM_CACHE[key]


def make_in_maps(inputs, NT, R):
    """Shard full inputs into per-core input maps (host-side, numpy)."""
    KT = NT // P
    RT = R // P
    H2M = H2 // P

    x = np.asarray(inputs["x"], np.float32)
    adj = np.asarray(inputs["adj_t"], np.float32)
    adj2 = np.asarray(inputs["adj_t2"], np.float32)
    we = np.asarray(inputs["w_embed"], np.float32)
    be = np.asarray(inputs["b_embed"], np.float32)
    gam = np.asarray(inputs["bn_gamma"], np.float32)
    bet = np.asarray(inputs["bn_beta"], np.float32)
    wf = np.asarray(inputs["w_fin"], np.float32)
    bf = np.asarray(inputs["b_fin"], np.float32)

    # binary adjacency factor + degree scalings (A = dis*Abin*dis exactly)
    ab = (adj > 0)
    ab2 = (adj2 > 0)
    d1 = ab.sum(1).astype(np.float32)
    d2 = ab2.sum(1).astype(np.float32)
    dis1 = np.where(d1 > 0, 1.0 / np.sqrt(np.maximum(d1, 1e-12)), 0.0
                    ).astype(np.float32)
    dis2 = np.where(d2 > 0, 1.0 / np.sqrt(np.maximum(d2, 1e-12)), 0.0
                    ).astype(np.float32)
    rs1 = adj.sum(1).astype(np.float32)    # exact rowsums of normalized A
    rs2 = adj2.sum(1).astype(np.float32)

    # host estimate of h and of column means of z (for conv2 centering),
    # plus rms-based power-of-2 scales for the fp8 operands
    h_host = np.maximum(x @ we.T + be, 0.0).astype(np.float32)
    mz = np.concatenate([(rs1 / NT) @ h_host, (rs2 / NT) @ h_host]
                        ).astype(np.float32)
    samp = np.arange(0, NT, 16)
    z_s = np.concatenate([adj[samp] @ h_host, adj2[samp] @ h_host], axis=1)
    zc_s = z_s - mz

    def pow2scale(v):
        r = np.sqrt(np.mean(v.astype(np.float64) ** 2))
        return float(2.0 ** np.round(np.log2(1.0 / max(r, 1e-30))))

    ta = pow2scale(dis1[samp, None] * zc_s)
    tb = pow2scale(dis2[samp, None] * zc_s)

    xTf_h = np.ascontiguousarray(x.T).astype(np.float16)
    wTe_h = np.ascontiguousarray(we.T).astype(np.float16)
    be_h = np.ascontiguousarray(be.reshape(H // P, P).T).astype(np.float32)
    bebc16_h = np.ascontiguousarray(
        np.broadcast_to(be[None, :], (P, H))).astype(np.float16)
    wTf_h = np.ascontiguousarray(wf.T).astype(np.float16)
    bff_h = np.ascontiguousarray(bf[:, None]).astype(np.float32)
    gam_h = np.ascontiguousarray(gam.reshape(H2M, P).T).astype(np.float32)
    bet_h = np.ascontiguousarray(bet.reshape(H2M, P).T).astype(np.float32)
    edis1_h = np.ascontiguousarray(dis1.reshape(KT, P).T).astype(np.float32)
    edis2_h = np.ascontiguousarray(dis2.reshape(KT, P).T).astype(np.float32)
    mzneg_h = np.ascontiguousarray((-mz).reshape(H2M, P).T).astype(np.float32)
    mzpos_h = np.ascontiguousarray(mz.reshape(H2M, P).T).astype(np.float32)

    in_maps = []
    for r in range(NCORES):
        rows = slice(r * R, (r + 1) * R)
        rsA_h = np.ascontiguousarray(
            np.broadcast_to(rs1[rows][None, :], (O, R))).astype(np.float32)
        rsA2_h = np.ascontiguousarray(
            np.broadcast_to(rs2[rows][None, :], (O, R))).astype(np.float32)
        in_maps.append({
            "xTf": xTf_h,
            "xT": np.ascontiguousarray(x[rows].T).astype(np.float16),
            "adjT": np.ascontiguousarray(ab[:, rows]).astype(NP_F8),
            "adjT2": np.ascontiguousarray(ab2[:, rows]).astype(NP_F8),
            "wTe": wTe_h, "be": be_h, "bebc16": bebc16_h, "wTf": wTf_h,
            "bff": bff_h, "gam": gam_h, "bet": bet_h,
            "edis1": edis1_h, "edis2": edis2_h,
            "zdis1": np.ascontiguousarray(
                (ta * dis1[rows]).reshape(RT, P).T).astype(np.float32),
            "zdis2": np.ascontiguousarray(
                (tb * dis2[rows]).reshape(RT, P).T).astype(np.float32),
            "mzneg": mzneg_h, "mzpos": mzpos_h,
            "dbc1": np.ascontiguousarray(
                np.broadcast_to(dis1[rows][None, :], (P, R))
            ).astype(np.float32),
            "dbc2": np.ascontiguousarray(
                np.broadcast_to(dis2[rows][None, :], (P, R))
            ).astype(np.float32),
            "ubc1": np.ascontiguousarray(
                np.broadcast_to((dis1[rows] / ta)[None, :], (P, R))
            ).astype(np.float32),
            "ubc2": np.ascontiguousarray(
                np.broadcast_to((dis2[rows] / tb)[None, :], (P, R))
            ).astype(np.float32),
            "rs1bc": np.ascontiguousarray(
                np.broadcast_to(rs1[rows][None, :], (P, R))
            ).astype(np.float32),
            "rs2bc": np.ascontiguousarray(
                np.broadcast_to(rs2[rows][None, :], (P, R))
            ).astype(np.float32),
            "rsA": rsA_h, "rsA2": rsA2_h,
        })
    return in_maps


def kernel(**inputs):
    NT, R = FULL_CFG["NT"], FULL_CFG["R"]
    nc = _get_program(NT, R)
    in_maps = make_in_maps(inputs, NT, R)
    res = run_bass_kernel_spmd(nc, in_maps, core_ids=list(range(NCORES)))
    out = np.concatenate(
        [res.results[r]["out"] for r in range(NCORES)], axis=0)
    return out.astype(np.float32)
